# revision 14
# baseline (speedup 1.0000x reference)
"""Trainium2 Bass kernel for ExllamaLinear (int4 group-quantized 4096x4096 linear).

out[b,s,o] = x @ W + bias,  W[i,o] = (nib4[i,o] - z[g(i),o]) * s[g(i),o]

Strategy (8 NeuronCores, data-parallel over tokens):
  - Host: shard x rows (8192 tokens -> 1024/core) and pre-transpose each shard to
    a plane-permuted [IN, M] layout (i' = k*512 + r for nibble plane k, qweight
    row r) so the device-side nibble unpack produces contraction rows in the
    matching order. Quantized weights are replicated to all cores (~9 MB).
  - Device: unpack int4 planes with dual-op tensor_scalar (shift+and) on DVE,
    cast on ScalarE, scale by repeated scales (DVE mult), then accumulate
    out[m,o] = sum_i' xt[i',m] * (nib*s)[i',o] on the PE over 32 K-blocks.
    The zero-point term is folded into the same PSUM accumulation as one extra
    K=33 matmul with rows [-xsumT; ones] x [zs; bias], where xsumT[g,m] (the
    per-group token sums) is computed on the PE with indicator matrices.
  - Host: concatenate the 8 [1024, 4096] row-shards.
"""
import os
import numpy as np

import concourse.bass as bass
import concourse.tile as tile
from concourse import bacc, mybir
from concourse.bass_utils import run_bass_kernel_spmd

N_CORES = 8
B, S, IN, OUT = 4, 2048, 4096, 4096
GROUP_SIZE = 128
M_TOT = B * S                 # 8192 tokens
M = M_TOT // N_CORES          # 1024 tokens per core
G = IN // GROUP_SIZE          # 32 groups
R = IN // 8                   # 512 packed qweight rows
NB = R // 128                 # 4 row-blocks
NJ = IN // 128                # 32 contraction blocks (= 8 planes x NB)
OC = 512                      # output-column chunk (one PSUM bank)
NOC = OUT // OC               # 8 chunks
NMB = M // 128                # 8 token-blocks per core

f16 = mybir.dt.float16
f32 = mybir.dt.float32
i32 = mybir.dt.int32
op = mybir.AluOpType


def build_nc(m=M, in_=IN, out=OUT):
    """Build the per-core Bass program. All shapes hardcoded for the full
    problem by default; smaller values only for simulator checks."""
    g = in_ // GROUP_SIZE
    r = in_ // 8
    nb = r // 128
    nj = in_ // 128
    noc = out // OC
    nmb = m // 128

    nc = bacc.Bacc("TRN2", target_bir_lowering=False, debug=False)

    xt_d = nc.dram_tensor("xt", [in_, m], f16, kind="ExternalInput")
    qw_d = nc.dram_tensor("qw", [r, out], i32, kind="ExternalInput")
    qz_d = nc.dram_tensor("qz", [g, out // 8], i32, kind="ExternalInput")
    sc_d = nc.dram_tensor("sc", [g, out], f16, kind="ExternalInput")
    sexp_d = nc.dram_tensor("sexp", [r, out], f16, kind="ExternalInput")
    bias_d = nc.dram_tensor("bias1", [1, out], f32, kind="ExternalInput")
    gind_d = nc.dram_tensor("gind", [nb * 128, g], f16, kind="ExternalInput")
    out_d = nc.dram_tensor("out", [m, out], f16, kind="ExternalOutput")

    with tile.TileContext(nc) as tc:
        with (
            tc.tile_pool(name="persist", bufs=1) as pp,
            tc.tile_pool(name="work", bufs=1) as wp,
            tc.tile_pool(name="psum", bufs=1, space="PSUM") as psp,
        ):
            # ---- resident inputs -------------------------------------------
            # grouped DMAs on the scalar queue so downstream matmuls can start
            # as blocks land while the sync queue streams the weight chunks
            xt3 = pp.tile([128, nj, m], f16)
            jg = 4
            for j0 in range(0, nj, jg):
                nc.scalar.dma_start(
                    xt3[:, j0:j0 + jg, :],
                    xt_d[j0 * 128:(j0 + jg) * 128, :].rearrange(
                        "(j p) m -> p j m", p=128))
            gind3 = pp.tile([128, nb, g], f16)
            nc.sync.dma_start(gind3[:], gind_d.rearrange("(b p) g -> p b g", p=128))
            qz_sb = pp.tile([g, out // 8], i32)
            nc.sync.dma_start(qz_sb[:], qz_d[:])

            # ---- zero-point row block: czs = [-(z*s); 0-pad; bias] (fp32) --
            # Rows g..31 are zero pad so the correction matmul is always K=33
            # with 32-aligned partition bases (required by DVE/PE).
            czs = pp.tile([33, out], f32)
            nc.vector.memset(czs[:], 0.0)
            nc.sync.dma_start(czs[32:33, :], bias_d[:])

            # ---- xsumT via indicator matmuls -> combo = [-xsumT; 0; ones] --
            combo = pp.tile([33, m], f32)
            nc.vector.memset(combo[:], 0.0)
            nc.vector.memset(combo[32:33, :], 1.0)
            for mh in range((m + 511) // 512):
                mw = min(512, m - mh * 512)
                ps_xs = psp.tile([128, 512], f32, tag="ps", bufs=8,
                                 name="ps_xs")
                for j in range(nj):
                    nc.tensor.matmul(
                        ps_xs[0:g, :mw], gind3[:, j % nb, :],
                        xt3[:, j, mh * 512:mh * 512 + mw],
                        start=(j == 0), stop=(j == nj - 1))
                nc.vector.tensor_copy(
                    combo[0:g, mh * 512:mh * 512 + mw], ps_xs[0:g, :mw])

            # ---- main loop over output-column chunks -----------------------
            for ocb in range(noc):
                osl = slice(ocb * OC, (ocb + 1) * OC)

                # zero-point chunk: czs[0:g, osl] = -(z*s)
                z_ic = wp.tile([g, OC], i32, tag="z_ic", bufs=2)
                for k2 in range(8):
                    nc.vector.tensor_scalar(
                        out=z_ic[:, k2::8],
                        in0=qz_sb[:, ocb * (OC // 8):(ocb + 1) * (OC // 8)],
                        scalar1=4 * k2, scalar2=0xF,
                        op0=op.logical_shift_right, op1=op.bitwise_and)
                z_fc = wp.tile([g, OC], f16, tag="z_fc", bufs=2)
                nc.scalar.copy(z_fc[:], z_ic[:])
                scc = wp.tile([g, OC], f16, tag="scc", bufs=2)
                nc.sync.dma_start(scc[:], sc_d[:, osl])
                nc.vector.scalar_tensor_tensor(
                    out=czs[0:g, osl], in0=z_fc[:], scalar=-1.0, in1=scc[:],
                    op0=op.mult, op1=op.mult)

                qwc = wp.tile([128, nb, OC], i32, tag="qwc", bufs=1)
                nc.sync.dma_start(
                    qwc[:], qw_d[:, osl].rearrange("(b p) o -> p b o", p=128))
                sec_t = wp.tile([128, nb, OC], f16, tag="sec", bufs=2)
                nc.sync.dma_start(
                    sec_t[:], sexp_d[:, osl].rearrange("(b p) o -> p b o", p=128))

                # dequant 4 row-blocks at a time per nibble plane: 8 wide DVE
                # ops per chunk instead of 32 (fewer PE-side sem waits, better
                # per-op overhead amortization). w3[:, k*nb+bb, :] covers
                # contraction rows i' = 128*(k*nb+bb) + p.
                w3 = wp.tile([128, nj, OC], f16, tag="w3", bufs=2)
                for k in range(8):
                    nib4 = wp.tile([128, nb, OC], i32, tag="nib4", bufs=2)
                    nc.vector.tensor_scalar(
                        out=nib4[:], in0=qwc[:], scalar1=4 * k, scalar2=0xF,
                        op0=op.logical_shift_right, op1=op.bitwise_and)
                    nibf4 = wp.tile([128, nb, OC], f16, tag="nibf4", bufs=2)
                    nc.scalar.copy(nibf4[:], nib4[:])
                    nc.vector.tensor_tensor(
                        w3[:, k * nb:(k + 1) * nb, :], nibf4[:], sec_t[:],
                        op.mult)

                for mb in range(nmb):
                    msl = slice(mb * 128, (mb + 1) * 128)
                    ps = psp.tile([128, OC], f32, tag="ps", bufs=8)
                    for j in range(nj):
                        nc.tensor.matmul(ps[:], xt3[:, j, msl], w3[:, j, :],
                                         start=(j == 0), stop=False)
                    nc.tensor.matmul(ps[:], combo[:, msl], czs[:, osl],
                                     start=False, stop=True)
                    ot = wp.tile([128, OC], f16, tag="ot", bufs=6)
                    nc.scalar.copy(ot[:], ps[:])
                    nc.sync.dma_start(out_d[msl, osl], ot[:])

    nc.compile()
    return nc


def shard_inputs(x, qweight, qzeros, scales, bias, m=M, in_=IN, out=OUT,
                 n_cores=N_CORES):
    """Host-side sharding / relayout (pure data movement + 0/1 indicators)."""
    g = in_ // GROUP_SIZE
    r = in_ // 8
    nb = r // 128

    x2 = np.asarray(x, dtype=np.float16).reshape(-1, in_)
    qweight = np.ascontiguousarray(np.asarray(qweight, dtype=np.int32))
    qzeros = np.ascontiguousarray(np.asarray(qzeros, dtype=np.int32))
    scales = np.ascontiguousarray(np.asarray(scales, dtype=np.float16))
    bias1 = np.asarray(bias, dtype=np.float32).reshape(1, out)
    sexp = np.ascontiguousarray(np.repeat(scales, 16, axis=0))

    gind = np.zeros((nb * 128, g), dtype=np.float16)
    for bb in range(nb):
        for p in range(128):
            gind[bb * 128 + p, 8 * bb + p // 16] = 1.0

    in_maps = []
    for c in range(n_cores):
        xc = x2[c * m:(c + 1) * m]                      # [m, in]
        xt = np.ascontiguousarray(
            xc.reshape(m, r, 8).transpose(2, 1, 0).reshape(in_, m))
        in_maps.append({
            "xt": xt, "qw": qweight, "qz": qzeros, "sc": scales,
            "sexp": sexp, "bias1": bias1, "gind": gind,
        })
    return in_maps


_NC_CACHE = {}


def kernel(x, qweight, qzeros, scales, bias):
    if "nc" not in _NC_CACHE:
        _NC_CACHE["nc"] = build_nc()
    nc = _NC_CACHE["nc"]
    in_maps = shard_inputs(x, qweight, qzeros, scales, bias)
    res = run_bass_kernel_spmd(nc, in_maps, list(range(N_CORES)))
    out = np.concatenate([res.results[c]["out"] for c in range(N_CORES)], axis=0)
    return out.reshape(B, S, OUT).astype(np.float16)


# revision 17
# speedup vs baseline: 1.1024x; 1.1024x over previous
"""Trainium2 Bass kernel for ExllamaLinear (int4 group-quantized 4096x4096 linear).

out[b,s,o] = x @ W + bias,  W[i,o] = (nib4[i,o] - z[g(i),o]) * s[g(i),o]

Strategy (8 NeuronCores, data-parallel over tokens):
  - Host: shard x rows (8192 tokens -> 1024/core) and pre-transpose each shard to
    a plane-permuted [IN, M] layout (i' = k*512 + r for nibble plane k, qweight
    row r) so the device-side nibble unpack produces contraction rows in the
    matching order. Quantized weights are replicated to all cores (~9 MB).
  - Device: unpack int4 planes with dual-op tensor_scalar (shift+and) on DVE,
    cast on ScalarE, scale by repeated scales (DVE mult), then accumulate
    out[m,o] = sum_i' xt[i',m] * (nib*s)[i',o] on the PE over 32 K-blocks.
    The zero-point term is folded into the same PSUM accumulation as one extra
    K=33 matmul with rows [-xsumT; ones] x [zs; bias], where xsumT[g,m] (the
    per-group token sums) is computed on the PE with indicator matrices.
  - Host: concatenate the 8 [1024, 4096] row-shards.
"""
import os
import numpy as np

import concourse.bass as bass
import concourse.tile as tile
from concourse import bacc, mybir
from concourse.bass_utils import run_bass_kernel_spmd

N_CORES = 8
B, S, IN, OUT = 4, 2048, 4096, 4096
GROUP_SIZE = 128
M_TOT = B * S                 # 8192 tokens
M = M_TOT // N_CORES          # 1024 tokens per core
G = IN // GROUP_SIZE          # 32 groups
R = IN // 8                   # 512 packed qweight rows
NB = R // 128                 # 4 row-blocks
NJ = IN // 128                # 32 contraction blocks (= 8 planes x NB)
OC = 512                      # output-column chunk (one PSUM bank)
NOC = OUT // OC               # 8 chunks
NMB = M // 128                # 8 token-blocks per core

f16 = mybir.dt.float16
f32 = mybir.dt.float32
i32 = mybir.dt.int32
op = mybir.AluOpType


def build_nc(m=M, in_=IN, out=OUT):
    """Build the per-core Bass program. All shapes hardcoded for the full
    problem by default; smaller values only for simulator checks."""
    g = in_ // GROUP_SIZE
    r = in_ // 8
    nb = r // 128
    nj = in_ // 128
    noc = out // OC
    nmb = m // 128

    nc = bacc.Bacc("TRN2", target_bir_lowering=False, debug=False)

    xt_d = nc.dram_tensor("xt", [in_, m], f16, kind="ExternalInput")
    qw_d = nc.dram_tensor("qw", [r, out], i32, kind="ExternalInput")
    qz_d = nc.dram_tensor("qz", [g, out // 8], i32, kind="ExternalInput")
    sc_d = nc.dram_tensor("sc", [g, out], f16, kind="ExternalInput")
    sexp_d = nc.dram_tensor("sexp", [r, out], f16, kind="ExternalInput")
    bias_d = nc.dram_tensor("bias1", [1, out], f16, kind="ExternalInput")
    gind_d = nc.dram_tensor("gind", [nb * 128, g], f16, kind="ExternalInput")
    out_d = nc.dram_tensor("out", [m, out], f16, kind="ExternalOutput")

    with tile.TileContext(nc) as tc:
        with (
            tc.tile_pool(name="persist", bufs=1) as pp,
            tc.tile_pool(name="work", bufs=1) as wp,
            tc.tile_pool(name="psum", bufs=1, space="PSUM") as psp,
        ):
            # ---- resident inputs -------------------------------------------
            # grouped DMAs on the scalar queue so downstream matmuls can start
            # as blocks land while the sync queue streams the weight chunks
            xt3 = pp.tile([128, nj, m], f16)
            jg = 4
            for j0 in range(0, nj, jg):
                nc.scalar.dma_start(
                    xt3[:, j0:j0 + jg, :],
                    xt_d[j0 * 128:(j0 + jg) * 128, :].rearrange(
                        "(j p) m -> p j m", p=128))
            gind3 = pp.tile([128, nb, g], f16)
            nc.sync.dma_start(gind3[:], gind_d.rearrange("(b p) g -> p b g", p=128))
            qz_sb = pp.tile([g, out // 8], i32)
            nc.sync.dma_start(qz_sb[:], qz_d[:])

            # ---- zero-point row block: czs = [-(z*s); 0-pad; bias] (fp32) --
            # Rows g..31 are zero pad so the correction matmul is always K=33
            # with 32-aligned partition bases (required by DVE/PE).
            czs = pp.tile([33, out], f16)
            nc.vector.memset(czs[:], 0.0)
            nc.sync.dma_start(czs[32:33, :], bias_d[:])

            # ---- xsumT via indicator matmuls -> combo = [-xsumT; 0; ones] --
            combo = pp.tile([33, m], f16)
            nc.vector.memset(combo[:], 0.0)
            nc.vector.memset(combo[32:33, :], 1.0)
            for mh in range((m + 511) // 512):
                mw = min(512, m - mh * 512)
                ps_xs = psp.tile([128, 512], f32, tag="ps", bufs=8,
                                 name="ps_xs")
                for j in range(nj):
                    nc.tensor.matmul(
                        ps_xs[0:g, :mw], gind3[:, j % nb, :],
                        xt3[:, j, mh * 512:mh * 512 + mw],
                        start=(j == 0), stop=(j == nj - 1))
                nc.vector.tensor_copy(
                    combo[0:g, mh * 512:mh * 512 + mw], ps_xs[0:g, :mw])

            # ---- main loop over output-column chunks -----------------------
            for ocb in range(noc):
                osl = slice(ocb * OC, (ocb + 1) * OC)

                # zero-point chunk: czs[0:g, osl] = -(z*s)
                z_ic = wp.tile([g, OC], i32, tag="z_ic", bufs=2)
                for k2 in range(8):
                    nc.vector.tensor_scalar(
                        out=z_ic[:, k2::8],
                        in0=qz_sb[:, ocb * (OC // 8):(ocb + 1) * (OC // 8)],
                        scalar1=4 * k2, scalar2=0xF,
                        op0=op.logical_shift_right, op1=op.bitwise_and)
                z_fc = wp.tile([g, OC], f16, tag="z_fc", bufs=2)
                nc.scalar.copy(z_fc[:], z_ic[:])
                scc = wp.tile([g, OC], f16, tag="scc", bufs=2)
                nc.sync.dma_start(scc[:], sc_d[:, osl])
                nc.vector.scalar_tensor_tensor(
                    out=czs[0:g, osl], in0=z_fc[:], scalar=-1.0, in1=scc[:],
                    op0=op.mult, op1=op.mult)

                qwc = wp.tile([128, nb, OC], i32, tag="qwc", bufs=1)
                nc.sync.dma_start(
                    qwc[:], qw_d[:, osl].rearrange("(b p) o -> p b o", p=128))
                sec_t = wp.tile([128, nb, OC], f16, tag="sec", bufs=2)
                nc.sync.dma_start(
                    sec_t[:], sexp_d[:, osl].rearrange("(b p) o -> p b o", p=128))

                # dequant 4 row-blocks at a time per nibble plane: 8 wide DVE
                # ops per chunk instead of 32 (fewer PE-side sem waits, better
                # per-op overhead amortization). w3[:, k*nb+bb, :] covers
                # contraction rows i' = 128*(k*nb+bb) + p.
                w3 = wp.tile([128, nj, OC], f16, tag="w3", bufs=2)
                for k in range(8):
                    nib4 = wp.tile([128, nb, OC], i32, tag="nib4", bufs=2)
                    nc.vector.tensor_scalar(
                        out=nib4[:], in0=qwc[:], scalar1=4 * k, scalar2=0xF,
                        op0=op.logical_shift_right, op1=op.bitwise_and)
                    nibf4 = wp.tile([128, nb, OC], f16, tag="nibf4", bufs=2)
                    nc.scalar.copy(nibf4[:], nib4[:])
                    nc.vector.tensor_tensor(
                        w3[:, k * nb:(k + 1) * nb, :], nibf4[:], sec_t[:],
                        op.mult)

                for mb in range(nmb):
                    msl = slice(mb * 128, (mb + 1) * 128)
                    ps = psp.tile([128, OC], f32, tag="ps", bufs=8)
                    for j in range(nj):
                        nc.tensor.matmul(ps[:], xt3[:, j, msl], w3[:, j, :],
                                         start=(j == 0), stop=False)
                    nc.tensor.matmul(ps[:], combo[:, msl], czs[:, osl],
                                     start=False, stop=True)
                    ot = wp.tile([128, OC], f16, tag="ot", bufs=6)
                    nc.scalar.copy(ot[:], ps[:])
                    nc.sync.dma_start(out_d[msl, osl], ot[:])

    nc.compile()
    return nc


def shard_inputs(x, qweight, qzeros, scales, bias, m=M, in_=IN, out=OUT,
                 n_cores=N_CORES):
    """Host-side sharding / relayout (pure data movement + 0/1 indicators)."""
    g = in_ // GROUP_SIZE
    r = in_ // 8
    nb = r // 128

    x2 = np.asarray(x, dtype=np.float16).reshape(-1, in_)
    qweight = np.ascontiguousarray(np.asarray(qweight, dtype=np.int32))
    qzeros = np.ascontiguousarray(np.asarray(qzeros, dtype=np.int32))
    scales = np.ascontiguousarray(np.asarray(scales, dtype=np.float16))
    bias1 = np.asarray(bias, dtype=np.float16).reshape(1, out)
    sexp = np.ascontiguousarray(np.repeat(scales, 16, axis=0))

    gind = np.zeros((nb * 128, g), dtype=np.float16)
    for bb in range(nb):
        for p in range(128):
            gind[bb * 128 + p, 8 * bb + p // 16] = 1.0

    in_maps = []
    for c in range(n_cores):
        xc = x2[c * m:(c + 1) * m]                      # [m, in]
        xt = np.ascontiguousarray(
            xc.reshape(m, r, 8).transpose(2, 1, 0).reshape(in_, m))
        in_maps.append({
            "xt": xt, "qw": qweight, "qz": qzeros, "sc": scales,
            "sexp": sexp, "bias1": bias1, "gind": gind,
        })
    return in_maps


_NC_CACHE = {}


def kernel(x, qweight, qzeros, scales, bias):
    if "nc" not in _NC_CACHE:
        _NC_CACHE["nc"] = build_nc()
    nc = _NC_CACHE["nc"]
    in_maps = shard_inputs(x, qweight, qzeros, scales, bias)
    res = run_bass_kernel_spmd(nc, in_maps, list(range(N_CORES)))
    out = np.concatenate([res.results[c]["out"] for c in range(N_CORES)], axis=0)
    return out.reshape(B, S, OUT).astype(np.float16)


# revision 19
# speedup vs baseline: 1.1139x; 1.0105x over previous
"""Trainium2 Bass kernel for ExllamaLinear (int4 group-quantized 4096x4096 linear).

out[b,s,o] = x @ W + bias,  W[i,o] = (nib4[i,o] - z[g(i),o]) * s[g(i),o]

Strategy (8 NeuronCores, data-parallel over tokens):
  - Host: shard x rows (8192 tokens -> 1024/core) and pre-transpose each shard to
    a plane-permuted [IN, M] layout (i' = k*512 + r for nibble plane k, qweight
    row r) so the device-side nibble unpack produces contraction rows in the
    matching order. Quantized weights are replicated to all cores (~9 MB).
  - Device: unpack int4 planes with dual-op tensor_scalar (shift+and) on DVE,
    cast on ScalarE, scale by repeated scales (DVE mult), then accumulate
    out[m,o] = sum_i' xt[i',m] * (nib*s)[i',o] on the PE over 32 K-blocks.
    The zero-point term is folded into the same PSUM accumulation as one extra
    K=33 matmul with rows [-xsumT; ones] x [zs; bias], where xsumT[g,m] (the
    per-group token sums) is computed on the PE with indicator matrices.
  - Host: concatenate the 8 [1024, 4096] row-shards.
"""
import os
import numpy as np

import concourse.bass as bass
import concourse.tile as tile
from concourse import bacc, mybir
from concourse.bass_utils import run_bass_kernel_spmd

N_CORES = 8
B, S, IN, OUT = 4, 2048, 4096, 4096
GROUP_SIZE = 128
M_TOT = B * S                 # 8192 tokens
M = M_TOT // N_CORES          # 1024 tokens per core
G = IN // GROUP_SIZE          # 32 groups
R = IN // 8                   # 512 packed qweight rows
NB = R // 128                 # 4 row-blocks
NJ = IN // 128                # 32 contraction blocks (= 8 planes x NB)
OC = 512                      # output-column chunk (one PSUM bank)
NOC = OUT // OC               # 8 chunks
NMB = M // 128                # 8 token-blocks per core

f16 = mybir.dt.float16
f32 = mybir.dt.float32
i32 = mybir.dt.int32
op = mybir.AluOpType


def build_nc(m=M, in_=IN, out=OUT):
    """Build the per-core Bass program. All shapes hardcoded for the full
    problem by default; smaller values only for simulator checks."""
    g = in_ // GROUP_SIZE
    r = in_ // 8
    nb = r // 128
    nj = in_ // 128
    noc = out // OC
    nmb = m // 128

    nc = bacc.Bacc("TRN2", target_bir_lowering=False, debug=False)

    xt_d = nc.dram_tensor("xt", [in_, m], f16, kind="ExternalInput")
    qw_d = nc.dram_tensor("qw", [r, out], i32, kind="ExternalInput")
    qz_d = nc.dram_tensor("qz", [g, out // 8], i32, kind="ExternalInput")
    sc_d = nc.dram_tensor("sc", [g, out], f16, kind="ExternalInput")
    sexp_d = nc.dram_tensor("sexp", [r, out], f16, kind="ExternalInput")
    bias_d = nc.dram_tensor("bias1", [1, out], f16, kind="ExternalInput")
    gind_d = nc.dram_tensor("gind", [nb * 128, g], f16, kind="ExternalInput")
    out_d = nc.dram_tensor("out", [m, out], f16, kind="ExternalOutput")

    with tile.TileContext(nc) as tc:
        with (
            tc.tile_pool(name="persist", bufs=1) as pp,
            tc.tile_pool(name="work", bufs=1) as wp,
            tc.tile_pool(name="psum", bufs=1, space="PSUM") as psp,
        ):
            # ---- resident inputs -------------------------------------------
            # grouped DMAs on the scalar queue so downstream matmuls can start
            # as blocks land while the sync queue streams the weight chunks
            xt3 = pp.tile([128, nj, m], f16)
            jg = 4
            for j0 in range(0, nj, jg):
                nc.scalar.dma_start(
                    xt3[:, j0:j0 + jg, :],
                    xt_d[j0 * 128:(j0 + jg) * 128, :].rearrange(
                        "(j p) m -> p j m", p=128))
            gind3 = pp.tile([128, nb, g], f16)
            nc.sync.dma_start(gind3[:], gind_d.rearrange("(b p) g -> p b g", p=128))
            qz_sb = pp.tile([g, out // 8], i32)
            nc.sync.dma_start(qz_sb[:], qz_d[:])

            # ---- zero-point row block: czs = [-(z*s); 0-pad; bias] (fp32) --
            # Rows g..31 are zero pad so the correction matmul is always K=33
            # with 32-aligned partition bases (required by DVE/PE).
            czs = pp.tile([33, out], f16)
            nc.vector.memset(czs[:], 0.0)
            nc.sync.dma_start(czs[32:33, :], bias_d[:])

            # ---- combo = [xsumT; 0; ones] (filled during first chunk) ------
            combo = pp.tile([33, m], f16)
            nc.vector.memset(combo[:], 0.0)
            nc.vector.memset(combo[32:33, :], 1.0)
            nmh = (m + 511) // 512
            ps_xs_tiles = [
                psp.tile([128, 512], f32, tag="ps", bufs=8, name=f"ps_xs{mh}")
                for mh in range(nmh)]

            # ---- main loop over output-column chunks -----------------------
            for ocb in range(noc):
                osl = slice(ocb * OC, (ocb + 1) * OC)

                # zero-point chunk: czs[0:g, osl] = -(z*s)
                z_ic = wp.tile([g, OC], i32, tag="z_ic", bufs=2)
                for k2 in range(8):
                    nc.vector.tensor_scalar(
                        out=z_ic[:, k2::8],
                        in0=qz_sb[:, ocb * (OC // 8):(ocb + 1) * (OC // 8)],
                        scalar1=4 * k2, scalar2=0xF,
                        op0=op.logical_shift_right, op1=op.bitwise_and)
                z_fc = wp.tile([g, OC], f16, tag="z_fc", bufs=2)
                nc.scalar.copy(z_fc[:], z_ic[:])
                scc = wp.tile([g, OC], f16, tag="scc", bufs=2)
                nc.sync.dma_start(scc[:], sc_d[:, osl])
                nc.vector.scalar_tensor_tensor(
                    out=czs[0:g, osl], in0=z_fc[:], scalar=-1.0, in1=scc[:],
                    op0=op.mult, op1=op.mult)

                qwc = wp.tile([128, nb, OC], i32, tag="qwc", bufs=1)
                nc.sync.dma_start(
                    qwc[:], qw_d[:, osl].rearrange("(b p) o -> p b o", p=128))
                sec_t = wp.tile([128, nb, OC], f16, tag="sec", bufs=2)
                nc.sync.dma_start(
                    sec_t[:], sexp_d[:, osl].rearrange("(b p) o -> p b o", p=128))

                # dequant 4 row-blocks at a time per nibble plane: 8 wide DVE
                # ops per chunk instead of 32 (fewer PE-side sem waits, better
                # per-op overhead amortization). w3[:, k*nb+bb, :] covers
                # contraction rows i' = 128*(k*nb+bb) + p.
                w3 = wp.tile([128, nj, OC], f16, tag="w3", bufs=2)
                for k in range(8):
                    nib4 = wp.tile([128, nb, OC], i32, tag="nib4", bufs=2)
                    nc.vector.tensor_scalar(
                        out=nib4[:], in0=qwc[:], scalar1=4 * k, scalar2=0xF,
                        op0=op.logical_shift_right, op1=op.bitwise_and)
                    nibf4 = wp.tile([128, nb, OC], f16, tag="nibf4", bufs=2)
                    nc.scalar.copy(nibf4[:], nib4[:])
                    nc.vector.tensor_tensor(
                        w3[:, k * nb:(k + 1) * nb, :], nibf4[:], sec_t[:],
                        op.mult)

                for mb in range(nmb):
                    msl = slice(mb * 128, (mb + 1) * 128)
                    ps = psp.tile([128, OC], f32, tag="ps", bufs=8)
                    for j in range(nj):
                        if ocb == 0 and mb == 0:
                            # xsumT matmuls interleaved into the DMA-paced
                            # fill window: 3 MMs per arriving xt block
                            for mh in range(nmh):
                                mw = min(512, m - mh * 512)
                                nc.tensor.matmul(
                                    ps_xs_tiles[mh][0:g, :mw],
                                    gind3[:, j % nb, :],
                                    xt3[:, j, mh * 512:mh * 512 + mw],
                                    start=(j == 0), stop=(j == nj - 1))
                        nc.tensor.matmul(ps[:], xt3[:, j, msl], w3[:, j, :],
                                         start=(j == 0), stop=False)
                    if ocb == 0 and mb == 0:
                        for mh in range(nmh):
                            mw = min(512, m - mh * 512)
                            nc.vector.tensor_copy(
                                combo[0:g, mh * 512:mh * 512 + mw],
                                ps_xs_tiles[mh][0:g, :mw])
                    nc.tensor.matmul(ps[:], combo[:, msl], czs[:, osl],
                                     start=False, stop=True)
                    ot = wp.tile([128, OC], f16, tag="ot", bufs=6)
                    nc.scalar.copy(ot[:], ps[:])
                    nc.sync.dma_start(out_d[msl, osl], ot[:])

    nc.compile()
    return nc


def shard_inputs(x, qweight, qzeros, scales, bias, m=M, in_=IN, out=OUT,
                 n_cores=N_CORES):
    """Host-side sharding / relayout (pure data movement + 0/1 indicators)."""
    g = in_ // GROUP_SIZE
    r = in_ // 8
    nb = r // 128

    x2 = np.asarray(x, dtype=np.float16).reshape(-1, in_)
    qweight = np.ascontiguousarray(np.asarray(qweight, dtype=np.int32))
    qzeros = np.ascontiguousarray(np.asarray(qzeros, dtype=np.int32))
    scales = np.ascontiguousarray(np.asarray(scales, dtype=np.float16))
    bias1 = np.asarray(bias, dtype=np.float16).reshape(1, out)
    sexp = np.ascontiguousarray(np.repeat(scales, 16, axis=0))

    gind = np.zeros((nb * 128, g), dtype=np.float16)
    for bb in range(nb):
        for p in range(128):
            gind[bb * 128 + p, 8 * bb + p // 16] = 1.0

    in_maps = []
    for c in range(n_cores):
        xc = x2[c * m:(c + 1) * m]                      # [m, in]
        xt = np.ascontiguousarray(
            xc.reshape(m, r, 8).transpose(2, 1, 0).reshape(in_, m))
        in_maps.append({
            "xt": xt, "qw": qweight, "qz": qzeros, "sc": scales,
            "sexp": sexp, "bias1": bias1, "gind": gind,
        })
    return in_maps


_NC_CACHE = {}


def kernel(x, qweight, qzeros, scales, bias):
    if "nc" not in _NC_CACHE:
        _NC_CACHE["nc"] = build_nc()
    nc = _NC_CACHE["nc"]
    in_maps = shard_inputs(x, qweight, qzeros, scales, bias)
    res = run_bass_kernel_spmd(nc, in_maps, list(range(N_CORES)))
    out = np.concatenate([res.results[c]["out"] for c in range(N_CORES)], axis=0)
    return out.reshape(B, S, OUT).astype(np.float16)


# revision 20
# speedup vs baseline: 1.1422x; 1.0254x over previous
"""Trainium2 Bass kernel for ExllamaLinear (int4 group-quantized 4096x4096 linear).

out[b,s,o] = x @ W + bias,  W[i,o] = (nib4[i,o] - z[g(i),o]) * s[g(i),o]

Strategy (8 NeuronCores, data-parallel over tokens):
  - Host: shard x rows (8192 tokens -> 1024/core) and pre-transpose each shard to
    a plane-permuted [IN, M] layout (i' = k*512 + r for nibble plane k, qweight
    row r) so the device-side nibble unpack produces contraction rows in the
    matching order. Quantized weights are replicated to all cores (~9 MB).
  - Device: unpack int4 planes with dual-op tensor_scalar (shift+and) on DVE,
    cast on ScalarE, scale by repeated scales (DVE mult), then accumulate
    out[m,o] = sum_i' xt[i',m] * (nib*s)[i',o] on the PE over 32 K-blocks.
    The zero-point term is folded into the same PSUM accumulation as one extra
    K=33 matmul with rows [-xsumT; ones] x [zs; bias], where xsumT[g,m] (the
    per-group token sums) is computed on the PE with indicator matrices.
  - Host: concatenate the 8 [1024, 4096] row-shards.
"""
import os
import numpy as np

import concourse.bass as bass
import concourse.tile as tile
from concourse import bacc, mybir
from concourse.bass_utils import run_bass_kernel_spmd

N_CORES = 8
B, S, IN, OUT = 4, 2048, 4096, 4096
GROUP_SIZE = 128
M_TOT = B * S                 # 8192 tokens
M = M_TOT // N_CORES          # 1024 tokens per core
G = IN // GROUP_SIZE          # 32 groups
R = IN // 8                   # 512 packed qweight rows
NB = R // 128                 # 4 row-blocks
NJ = IN // 128                # 32 contraction blocks (= 8 planes x NB)
OC = 512                      # output-column chunk (one PSUM bank)
NOC = OUT // OC               # 8 chunks
NMB = M // 128                # 8 token-blocks per core

f16 = mybir.dt.float16
f32 = mybir.dt.float32
i32 = mybir.dt.int32
op = mybir.AluOpType


def build_nc(m=M, in_=IN, out=OUT):
    """Build the per-core Bass program. All shapes hardcoded for the full
    problem by default; smaller values only for simulator checks."""
    g = in_ // GROUP_SIZE
    r = in_ // 8
    nb = r // 128
    nj = in_ // 128
    noc = out // OC
    nmb = m // 128

    nc = bacc.Bacc("TRN2", target_bir_lowering=False, debug=False)

    xt_d = nc.dram_tensor("xt", [in_, m], f16, kind="ExternalInput")
    qw_d = nc.dram_tensor("qw", [r, out], i32, kind="ExternalInput")
    qz_d = nc.dram_tensor("qz", [g, out // 8], i32, kind="ExternalInput")
    sc_d = nc.dram_tensor("sc", [g, out], f16, kind="ExternalInput")
    sexp_d = nc.dram_tensor("sexp", [r, out], f16, kind="ExternalInput")
    bias_d = nc.dram_tensor("bias1", [1, out], f16, kind="ExternalInput")
    gind_d = nc.dram_tensor("gind", [nb * 128, g], f16, kind="ExternalInput")
    out_d = nc.dram_tensor("out", [m, out], f16, kind="ExternalOutput")

    with tile.TileContext(nc) as tc:
        with (
            tc.tile_pool(name="persist", bufs=1) as pp,
            tc.tile_pool(name="work", bufs=1) as wp,
            tc.tile_pool(name="psum", bufs=1, space="PSUM") as psp,
        ):
            # ---- resident inputs -------------------------------------------
            # grouped DMAs on the scalar queue so downstream matmuls can start
            # as blocks land while the sync queue streams the weight chunks
            xt3 = pp.tile([128, nj, m], f16)
            jg = 4
            for j0 in range(0, nj, jg):
                nc.scalar.dma_start(
                    xt3[:, j0:j0 + jg, :],
                    xt_d[j0 * 128:(j0 + jg) * 128, :].rearrange(
                        "(j p) m -> p j m", p=128))
            gind3 = pp.tile([128, nb, g], f16)
            nc.sync.dma_start(gind3[:], gind_d.rearrange("(b p) g -> p b g", p=128))
            qz_sb = pp.tile([g, out // 8], i32)
            nc.sync.dma_start(qz_sb[:], qz_d[:])

            # ---- zero-point row block: czs = [-(z*s); 0-pad; bias] (fp32) --
            # Rows g..31 are zero pad so the correction matmul is always K=33
            # with 32-aligned partition bases (required by DVE/PE).
            czs = pp.tile([128, out], f16)
            nc.vector.memset(czs[:], 0.0)
            nc.sync.dma_start(czs[32:33, :], bias_d[:])

            # ---- combo = [xsumT; 0; ones] (filled during first chunk) ------
            combo = pp.tile([128, m], f16)
            nc.vector.memset(combo[:], 0.0)
            nc.vector.memset(combo[32:33, :], 1.0)
            nmh = (m + 511) // 512
            ps_xs_tiles = [
                psp.tile([128, 512], f32, tag="ps", bufs=8, name=f"ps_xs{mh}")
                for mh in range(nmh)]

            # ---- main loop over output-column chunks -----------------------
            for ocb in range(noc):
                osl = slice(ocb * OC, (ocb + 1) * OC)

                # zero-point chunk: czs[0:g, osl] = -(z*s)
                z_ic = wp.tile([g, OC], i32, tag="z_ic", bufs=2)
                for k2 in range(8):
                    nc.vector.tensor_scalar(
                        out=z_ic[:, k2::8],
                        in0=qz_sb[:, ocb * (OC // 8):(ocb + 1) * (OC // 8)],
                        scalar1=4 * k2, scalar2=0xF,
                        op0=op.logical_shift_right, op1=op.bitwise_and)
                z_fc = wp.tile([g, OC], f16, tag="z_fc", bufs=2)
                nc.scalar.copy(z_fc[:], z_ic[:])
                scc = wp.tile([g, OC], f16, tag="scc", bufs=2)
                nc.sync.dma_start(scc[:], sc_d[:, osl])
                nc.vector.scalar_tensor_tensor(
                    out=czs[0:g, osl], in0=z_fc[:], scalar=-1.0, in1=scc[:],
                    op0=op.mult, op1=op.mult)

                qwc = wp.tile([128, nb, OC], i32, tag="qwc", bufs=1)
                nc.sync.dma_start(
                    qwc[:], qw_d[:, osl].rearrange("(b p) o -> p b o", p=128))
                sec_t = wp.tile([128, nb, OC], f16, tag="sec", bufs=2)
                nc.sync.dma_start(
                    sec_t[:], sexp_d[:, osl].rearrange("(b p) o -> p b o", p=128))

                # dequant 4 row-blocks at a time per nibble plane: 8 wide DVE
                # ops per chunk instead of 32 (fewer PE-side sem waits, better
                # per-op overhead amortization). w3[:, k*nb+bb, :] covers
                # contraction rows i' = 128*(k*nb+bb) + p.
                w3 = wp.tile([128, nj, OC], f16, tag="w3", bufs=2)
                for k in range(8):
                    nib4 = wp.tile([128, nb, OC], i32, tag="nib4", bufs=2)
                    nc.vector.tensor_scalar(
                        out=nib4[:], in0=qwc[:], scalar1=4 * k, scalar2=0xF,
                        op0=op.logical_shift_right, op1=op.bitwise_and)
                    nibf4 = wp.tile([128, nb, OC], f16, tag="nibf4", bufs=2)
                    nc.scalar.copy(nibf4[:], nib4[:])
                    nc.vector.tensor_tensor(
                        w3[:, k * nb:(k + 1) * nb, :], nibf4[:], sec_t[:],
                        op.mult)

                for mb in range(nmb):
                    msl = slice(mb * 128, (mb + 1) * 128)
                    ps = psp.tile([128, OC], f32, tag="ps", bufs=8)
                    for j in range(nj):
                        if ocb == 0 and mb == 0:
                            # xsumT matmuls interleaved into the DMA-paced
                            # fill window: 3 MMs per arriving xt block
                            for mh in range(nmh):
                                mw = min(512, m - mh * 512)
                                nc.tensor.matmul(
                                    ps_xs_tiles[mh][0:g, :mw],
                                    gind3[:, j % nb, :],
                                    xt3[:, j, mh * 512:mh * 512 + mw],
                                    start=(j == 0), stop=(j == nj - 1))
                        nc.tensor.matmul(ps[:], xt3[:, j, msl], w3[:, j, :],
                                         start=(j == 0), stop=False)
                    if ocb == 0 and mb == 0:
                        for mh in range(nmh):
                            mw = min(512, m - mh * 512)
                            nc.vector.tensor_copy(
                                combo[0:g, mh * 512:mh * 512 + mw],
                                ps_xs_tiles[mh][0:g, :mw])
                    nc.tensor.matmul(ps[:], combo[:, msl], czs[:, osl],
                                     start=False, stop=True)
                    ot = wp.tile([128, OC], f16, tag="ot", bufs=6)
                    nc.scalar.copy(ot[:], ps[:])
                    nc.sync.dma_start(out_d[msl, osl], ot[:])

    nc.compile()
    return nc


def shard_inputs(x, qweight, qzeros, scales, bias, m=M, in_=IN, out=OUT,
                 n_cores=N_CORES):
    """Host-side sharding / relayout (pure data movement + 0/1 indicators)."""
    g = in_ // GROUP_SIZE
    r = in_ // 8
    nb = r // 128

    x2 = np.asarray(x, dtype=np.float16).reshape(-1, in_)
    qweight = np.ascontiguousarray(np.asarray(qweight, dtype=np.int32))
    qzeros = np.ascontiguousarray(np.asarray(qzeros, dtype=np.int32))
    scales = np.ascontiguousarray(np.asarray(scales, dtype=np.float16))
    bias1 = np.asarray(bias, dtype=np.float16).reshape(1, out)
    sexp = np.ascontiguousarray(np.repeat(scales, 16, axis=0))

    gind = np.zeros((nb * 128, g), dtype=np.float16)
    for bb in range(nb):
        for p in range(128):
            gind[bb * 128 + p, 8 * bb + p // 16] = 1.0

    in_maps = []
    for c in range(n_cores):
        xc = x2[c * m:(c + 1) * m]                      # [m, in]
        xt = np.ascontiguousarray(
            xc.reshape(m, r, 8).transpose(2, 1, 0).reshape(in_, m))
        in_maps.append({
            "xt": xt, "qw": qweight, "qz": qzeros, "sc": scales,
            "sexp": sexp, "bias1": bias1, "gind": gind,
        })
    return in_maps


_NC_CACHE = {}


def kernel(x, qweight, qzeros, scales, bias):
    if "nc" not in _NC_CACHE:
        _NC_CACHE["nc"] = build_nc()
    nc = _NC_CACHE["nc"]
    in_maps = shard_inputs(x, qweight, qzeros, scales, bias)
    res = run_bass_kernel_spmd(nc, in_maps, list(range(N_CORES)))
    out = np.concatenate([res.results[c]["out"] for c in range(N_CORES)], axis=0)
    return out.reshape(B, S, OUT).astype(np.float16)
